# revision 23
# baseline (speedup 1.0000x reference)
"""Trainium2 Bass kernel: per-row bincount (BagOfWords) over 8 NeuronCores.

Problem: inputs int32 [16384, 200], values in [0, 1100); output f32
[16384, 1099] = per-row histogram over token ids 1..1099 (bin 0 dropped).

Strategy (pure data parallel): shard the batch over 8 cores (2048 rows
each). Per core, factorize each token id v = 32*a + b (a = v>>5 in
[0,35), b = v&31 in [0,32)) and compute the per-row histogram as a tiny
per-row matmul on the PE systolic array:

    psum[tb, ta] = sum_j onehot_b(b_j)[tb] * onehot_a(a_j)[ta]

with the contraction over token slots on the partition dim (k = 128 + 72).
Digit extraction is 2 DVE bit ops (shift/and, exact) + 2 Scalar casts to
bf16. Digit tensors are transposed to k-major via PE transpose (bf16);
one-hot planes are built in bf16 with per-digit is_equal compares split
between the Vector engine (DVE 4x perf mode) and the GPSIMD/Pool engine.
Tiles are processed in groups of [1, 2x7, 1] x 128 rows: the half-size
groups at both ends shorten pipeline fill (first matmul flush waits on a
half-size one-hot generation) and drain (final matmul burst + eviction
is half-size). Matmul emission is software-pipelined one group behind
generation. Per-row [32, 35] results are packed 4-across-partitions (PE
col-groups) x 8-across-free per PSUM bank, evicted in bulk to a bf16
stage on the Scalar engine, and written with one large contiguous DMA
per group to a flat bf16 output in stage order; the host decodes the
(row, digit) interleaving, applies the v = 32a+b column permutation,
drops bin 0 and bins >= 1100, and concatenates shards. All arithmetic
is exact (integer-valued bf16/f32; counts <= 200 < 256 are bf16-exact).
"""

import numpy as np
import ml_dtypes
from contextlib import ExitStack

import concourse.bass as bass
import concourse.tile as tile
from concourse import bacc, mybir
from concourse.bass_utils import run_bass_kernel_spmd

BF16 = mybir.dt.bfloat16
F32 = mybir.dt.float32
I32 = mybir.dt.int32
AluOp = mybir.AluOpType

N_CORES = 8
FULL_B = 16384
S = 200
NB, NA = 32, 35          # b = v & 31 (stationary planes), a = v >> 5 (moving)
V = NB * NA              # 1120 device bins; host drops 0 and 1100..1119
KA, KB = 128, 72
RG = 64

# one-hot plane assignment: Pool takes the first planes, the Scalar
# (Activation) engine takes the last SCALAR_A a-planes via the exact
# integer identity [a == ta] = relu(1 - |a - ta|), DVE takes the rest
POOL_A = 7               # a-planes 0..POOL_A-1 on Pool
POOL_B = 6               # b-planes 0..POOL_B-1 on Pool
SCALAR_A = 0             # a-planes NA-SCALAR_A..NA-1 on Scalar

# tile groups: singles at both ends shorten pipeline fill/drain
GROUPS = [[0]] + [[2 * i + 1, 2 * i + 2] for i in range(7)] + [[15]]


def _host_consts():
    ident = np.eye(128, dtype=np.float32).astype(ml_dtypes.bfloat16)
    return {"ident": np.ascontiguousarray(ident)}


def _emit_group_mms(nc, ps_tiles, oh3B, oh3A, stage, n_tiles, y, y_off):
    n_g = 2 * n_tiles    # 64-row psum groups in this tile group
    for g in range(n_g):
        r0 = g * RG
        ps = ps_tiles[g % 2]
        for r in range(RG):
            rr = r0 + r          # row within the group
            tile_half = rr // 128
            rloc = rr % 128
            base = 256 * tile_half
            s = r % 4
            q = (r // 4) % 8
            b2 = r // 32
            out_ap = ps[32 * s:32 * s + NB,
                        512 * b2 + NA * q:512 * b2 + NA * q + NA]
            nc.tensor.matmul(out_ap,
                             oh3B[:, base + rloc, :],
                             oh3A[:, base + rloc, :],
                             start=True, stop=False,
                             tile_position=(0, 32 * s))
            nc.tensor.matmul(out_ap,
                             oh3B[0:KB, base + 128 + rloc, :],
                             oh3A[0:KB, base + 128 + rloc, :],
                             start=False, stop=True,
                             tile_position=(0, 32 * s))

        nc.scalar.copy(
            stage[:, 560 * g:560 * (g + 1)].rearrange(
                "p (b c) -> p b c", c=280),
            ps[:].rearrange("p (b c) -> p b c", c=512)[:, :, 0:280])

        # per-psum-group DMA (1120B descriptors): each 64-row slab ships
        # as soon as its eviction lands, so the final transfer is small
        W = 560 * n_g
        nc.sync.dma_start(
            bass.AP(y, y_off + 560 * g, [[W, 128], [1, 560]]),
            stage[:, 560 * g:560 * (g + 1)])
    return y_off + 128 * W


def _kernel_body(ctx, tc, y, x, ident_d):
    nc = tc.nc

    const_pool = ctx.enter_context(tc.tile_pool(name="const", bufs=1))
    io_pool = ctx.enter_context(tc.tile_pool(name="io", bufs=5))
    dig_pool = ctx.enter_context(tc.tile_pool(name="dig", bufs=4))
    kt_pool = ctx.enter_context(tc.tile_pool(name="kt", bufs=3))
    oh_pool = ctx.enter_context(tc.tile_pool(name="oh", bufs=2))
    tp_psum = ctx.enter_context(tc.tile_pool(name="tp", bufs=2, space="PSUM"))
    mm_psum = ctx.enter_context(tc.tile_pool(name="mm", bufs=1, space="PSUM"))
    stage_pool = ctx.enter_context(tc.tile_pool(name="stage", bufs=2))
    # half-size (single-tile) groups at the ends: far-apart reuse, 1 buffer
    kt1_pool = ctx.enter_context(tc.tile_pool(name="kt1", bufs=1))
    oh1_pool = ctx.enter_context(tc.tile_pool(name="oh1", bufs=1))
    stage1_pool = ctx.enter_context(tc.tile_pool(name="stage1", bufs=1))
    sc_pool = ctx.enter_context(tc.tile_pool(name="sc", bufs=2))

    # Load constants once.
    c_id = const_pool.tile([128, 128], BF16, tag="c_id")
    nc.sync.dma_start(c_id[:], ident_d.ap())
    # per-partition bias columns for the Scalar-engine one-hot planes
    sc_biases = {}
    for ta in range(NA - SCALAR_A, NA):
        bias_t = const_pool.tile([128, 1], F32, tag=f"bias{ta}",
                                 name=f"bias{ta}")
        nc.gpsimd.memset(bias_t[:], float(-ta))
        sc_biases[ta] = bias_t

    # Persistent psum accumulators (2, used alternately). Matmuls cover all
    # 128 partitions (4 col groups) and the eviction only reads columns the
    # matmuls wrote, so no zero-init is needed.
    ps0 = mm_psum.tile([128, 1024], F32, tag="ps0")
    ps1 = mm_psum.tile([128, 1024], F32, tag="ps1")
    ps_tiles = [ps0, ps1]

    kt_seen = {}

    def prep_group(gi):
        """Load, digit-extract, transpose, and pack kt for one group."""
        tiles = GROUPS[gi]
        L = len(tiles)
        tag = f"L{L}"
        ktp = kt_pool if L == 2 else kt1_pool
        n_bufs = 3 if L == 2 else 1
        bT = ktp.tile([128, 256 * L], BF16, tag=f"bT{tag}", name=f"bT{gi}")
        aT = ktp.tile([128, 256 * L], BF16, tag=f"aT{tag}", name=f"aT{gi}")
        # zero the k-half-B padding rows once per fresh buffer (later
        # occupants never write rows 72:128 of the B columns, so the
        # zeros persist across buffer reuse)
        seen = kt_seen.get(tag, 0)
        kt_seen[tag] = seen + 1
        if seen < n_bufs:
            for h in range(L):
                o = 256 * h
                nc.scalar.memzero(bT[64:128, o + 128:o + 256])
                nc.scalar.memzero(aT[64:128, o + 128:o + 256])

        for h, t in enumerate(tiles):
            # ---- load + digit extraction ([128 rows, 200 seq]) ----
            xa = io_pool.tile([128, S], I32, tag="xa", name=f"xa{t}")
            in_eng = nc.sync if t % 2 == 0 else nc.scalar
            in_eng.dma_start(xa[:], x.ap()[t * 128:(t + 1) * 128, :])

            a32 = dig_pool.tile([128, S], I32, tag="a32", name=f"a32_{t}")
            nc.vector.tensor_scalar(a32[:], xa[:], 5, None,
                                    AluOp.arith_shift_right)
            b32 = dig_pool.tile([128, S], I32, tag="b32", name=f"b32_{t}")
            nc.vector.tensor_scalar(b32[:], xa[:], 31, None,
                                    AluOp.bitwise_and)
            a16 = dig_pool.tile([128, S], BF16, tag="a16", name=f"a16_{t}")
            nc.scalar.copy(a16[:], a32[:])
            b16 = dig_pool.tile([128, S], BF16, tag="b16", name=f"b16_{t}")
            nc.scalar.copy(b16[:], b32[:])

            # ---- transpose to k-major ----
            tp = tp_psum.tile([128, 512], BF16, tag="tp", name=f"tp{t}")
            nc.tensor.transpose(tp[:, 0:128], b16[:, 0:128], c_id[:])
            nc.tensor.transpose(tp[0:KB, 128:256], b16[:, 128:S], c_id[:])
            nc.tensor.transpose(tp[:, 256:384], a16[:, 0:128], c_id[:])
            nc.tensor.transpose(tp[0:KB, 384:512], a16[:, 128:S], c_id[:])

            o = 256 * h
            nc.scalar.copy(bT[:, o + 0:o + 128], tp[:, 0:128])
            nc.scalar.copy(bT[0:KB, o + 128:o + 256], tp[0:KB, 128:256])
            nc.scalar.copy(aT[:, o + 0:o + 128], tp[:, 256:384])
            nc.scalar.copy(aT[0:KB, o + 128:o + 256], tp[0:KB, 384:512])
        return aT, bT

    n_groups = len(GROUPS)
    preps = {0: prep_group(0)}
    if n_groups > 1:
        preps[1] = prep_group(1)
    pend = None
    y_off = 0
    for gi, tiles in enumerate(GROUPS):
        L = len(tiles)
        W = 256 * L
        tag = f"L{L}"
        # prep two groups ahead so the gen pacer never waits on kt
        if gi + 2 < n_groups:
            preps[gi + 2] = prep_group(gi + 2)
        aT, bT = preps.pop(gi)

        # ---- Scalar-engine one-hot planes for THIS group, emitted
        # before the previous group's evictions so they clear the
        # in-order Activation queue early (they only need aT) ----
        ohp = oh_pool if L == 2 else oh1_pool
        ohB = ohp.tile([128, NB * W], BF16, tag=f"ohB{tag}", name=f"ohB{gi}")
        ohA = ohp.tile([128, NA * W], BF16, tag=f"ohA{tag}", name=f"ohA{gi}")
        for ta in range(NA - SCALAR_A, NA):
            # exact integer identity: [a == ta] = relu(1 - |a - ta|)
            sc = sc_pool.tile([128, W], BF16, tag=f"sc{tag}",
                              name=f"sc{gi}_{ta}")
            nc.scalar.activation(sc[:], aT[:],
                                 mybir.ActivationFunctionType.Abs,
                                 bias=sc_biases[ta][:])
            nc.scalar.activation(ohA[:, ta * W:(ta + 1) * W], sc[:],
                                 mybir.ActivationFunctionType.Relu,
                                 bias=1.0, scale=-1.0)

        # ---- flush PREVIOUS group's matmuls ----
        if pend is not None:
            y_off = _emit_group_mms(nc, ps_tiles, *pend, y, y_off)

        # ---- one-hot generation over the group (W cols/op), split
        # between DVE (4x perf mode) and the Pool engine ----
        pool_a = POOL_A + (1 if gi % 2 == 1 else 0)  # alternating balance
        pool_b = POOL_B
        if gi in (5, 6, 7):
            pool_a += 1          # Pool has late-run headroom; load it up
        if gi == len(GROUPS) - 1:
            # tail group: keep Pool light so its slower per-op tail
            # doesn't delay the final matmul burst
            pool_a, pool_b = 3, 3
        for tb in range(NB):
            eng = nc.gpsimd if tb < pool_b else nc.vector
            eng.tensor_scalar(ohB[:, tb * W:(tb + 1) * W],
                              bT[:], float(tb), None, AluOp.is_equal)
        for ta in range(NA - SCALAR_A):
            eng = nc.gpsimd if ta < pool_a else nc.vector
            eng.tensor_scalar(ohA[:, ta * W:(ta + 1) * W],
                              aT[:], float(ta), None, AluOp.is_equal)
        stp = stage_pool if L == 2 else stage1_pool
        stage = stp.tile([128, 1120 * L], BF16, tag=f"stage{tag}",
                         name=f"stage{gi}")
        pend = (ohB[:].rearrange("p (c r) -> p r c", c=NB),
                ohA[:].rearrange("p (c r) -> p r c", c=NA),
                stage, L)
    _emit_group_mms(nc, ps_tiles, *pend, y, y_off)


def _build_program():
    B = FULL_B // N_CORES
    nc = bacc.Bacc("TRN2", target_bir_lowering=False, debug=False,
                   num_devices=N_CORES)
    x = nc.dram_tensor("x", [B, S], I32, kind="ExternalInput")
    ident = nc.dram_tensor("ident", [128, 128], BF16, kind="ExternalInput")
    # stage-order output, flat: per group, 128 stage rows x 560*2L cols
    y = nc.dram_tensor("y", [B * V], BF16, kind="ExternalOutput")
    with tile.TileContext(nc) as tc:
        with ExitStack() as ctx:
            _kernel_body(ctx, tc, y, x, ident)
    nc.compile()
    return nc


_program_cache = {}


def _get_program():
    if "nc" not in _program_cache:
        _program_cache["nc"] = _build_program()
    return _program_cache["nc"]


def _decode_host(y_dev: np.ndarray, B: int) -> np.ndarray:
    """flat stage dump -> [B, 1120] f32 (device-bin column order)."""
    out = np.empty((B, V), dtype=np.float32)
    off = 0
    row0 = 0
    for tiles in GROUPS:
        L = len(tiles)
        n_g = 2 * L
        W = 560 * n_g
        chunk = y_dev[off:off + 128 * W].reshape(4, 32, n_g, 2, 8, 35)
        #                       dims:          s   tb  g  b2  q  ta
        # row = row0 + 64g + 32b2 + 4q + s ; device col = 35*tb + ta
        chunk = chunk.transpose(2, 3, 4, 0, 1, 5)  # g,b2,q,s,tb,ta
        out[row0:row0 + 128 * L] = chunk.reshape(128 * L, V)
        off += 128 * W
        row0 += 128 * L
    return out


# column permutation: out bin v lives at device col 35*(v&31) + (v>>5)
_COLPERM = 35 * (np.arange(1, 1100) & 31) + (np.arange(1, 1100) >> 5)


def kernel(**inputs) -> np.ndarray:
    B = FULL_B // N_CORES
    x_full = np.ascontiguousarray(np.asarray(inputs["inputs"], dtype=np.int32))
    assert x_full.shape == (FULL_B, S), x_full.shape

    nc = _get_program()
    consts = _host_consts()
    in_maps = []
    for c in range(N_CORES):
        m = {"x": np.ascontiguousarray(x_full[c * B:(c + 1) * B])}
        m.update(consts)
        in_maps.append(m)

    res = run_bass_kernel_spmd(nc, in_maps, core_ids=list(range(N_CORES)))
    ys = [_decode_host(np.asarray(res.results[c]["y"]), B)
          for c in range(N_CORES)]
    full = np.concatenate(ys, axis=0)
    return np.ascontiguousarray(full[:, _COLPERM])
